# revision 9
# baseline (speedup 1.0000x reference)
"""Adaptive Computation Time kernel for 8 TRN2 NeuronCores.

Reference semantics: p = sigmoid(x @ W + b) is constant across the 20 ACT
steps, so the scan has a closed form per position:
  N_raw = floor(0.99/p) + 1          (first step n with n*p > 0.99)
  halted = N_raw <= 20
  e = min(floor(0.99/p), 20)          (exponent; = N-1 if halted else 20)
  state  = x * (1 - (1-p)^e * (halted ? e*p : 1))
  ponder = halted ? (e+1) + 1 - (e+1)*p : 20
with (1-p)^e = exp(-e * softplus(z)), z = x@W + b.

Sharding: data-parallel over batch (B=8 -> 1 batch element per core).
"""

import os
import sys

import numpy as np


def _ensure_paths():
    for p in (
        "/root/.axon_site",
        "/root/.axon_site/_ro/trn_rl_repo",
        "/root/.axon_site/_ro/pypackages",
        "/opt/trn_rl_repo",
        "/opt/pypackages",
    ):
        if os.path.isdir(p) and p not in sys.path:
            sys.path.append(p)


_ensure_paths()

import concourse.bass as bass  # noqa: E402
import concourse.bacc as bacc  # noqa: E402
import concourse.mybir as mybir  # noqa: E402
import concourse.tile as tile  # noqa: E402
from concourse.bass_utils import run_bass_kernel_spmd  # noqa: E402

B, S, D = 8, 4096, 1024
N_CORES = 8
ROWS = (B * S) // N_CORES  # 4096 rows per core
P = 128
NT = ROWS // P  # 32 row-tiles per core
G = 16  # tiles per closed-form group
THRESH = float(np.float32(0.99))
F32 = mybir.dt.float32
A = mybir.AluOpType
AF = mybir.ActivationFunctionType


def build_nc():
    nc = bacc.Bacc("TRN2", target_bir_lowering=False, debug=False)

    x_ext = nc.declare_dram_parameter("x", [ROWS, D], F32, isOutput=False)
    w_ext = nc.declare_dram_parameter("W", [P, D], F32, isOutput=False)  # replicated
    b_ext = nc.declare_dram_parameter("nb", [P, 1], F32, isOutput=False)  # -b replicated
    st_ext = nc.declare_dram_parameter("out_state", [ROWS, D], F32, isOutput=True)
    pd_ext = nc.declare_dram_parameter("out_ponder", [ROWS, 1], F32, isOutput=True)

    # ponder viewed as (p, t): row 128*t + p  ->  partition p, column t
    pd_view = pd_ext[:, :].rearrange("(t p) one -> p (t one)", p=P)

    with tile.TileContext(nc) as tc:
        with (
            tc.tile_pool(name="const", bufs=1) as cpool,
            tc.tile_pool(name="xdata", bufs=NT) as xpool,
            tc.tile_pool(name="scratch", bufs=2) as spool,
            tc.tile_pool(name="small", bufs=2) as smp,
        ):
            # --- W and -b arrive pre-replicated across partitions ---
            wb = cpool.tile([P, D], F32)
            nc.sync.dma_start(wb[:], w_ext[:, :])
            nbcol = cpool.tile([P, 1], F32)
            nc.sync.dma_start(nbcol[:], b_ext[:, :])

            for g in range(NT // G):
                zb = smp.tile([P, G], F32, tag="zb")
                xts = []
                for j in range(G):
                    t = g * G + j
                    xt = xpool.tile([P, D], F32, tag="xt")
                    nc.sync.dma_start(xt[:], x_ext[bass.ts(t, P), :])
                    xts.append(xt)
                    scr = spool.tile([P, D], F32, tag="scr")
                    nc.vector.affine_mul_reduce(
                        out=scr[:],
                        accum_out=zb[:, j : j + 1],
                        in0=xt[:],
                        in1=wb[:],
                        scale=1.0,
                        bias=0.0,
                    )

                # --- closed form on zb (P, G) ---
                # Single ACT table (natural_log_exp): Exp, Ln, Copy only.
                def st(tag):
                    return smp.tile([P, G], F32, tag=tag, name=tag)

                ez = st("ez")
                nc.scalar.activation(
                    ez[:], zb[:], AF.Exp, scale=-1.0, bias=nbcol[:]
                )  # e^-(z+b)
                den = st("den")
                nc.vector.tensor_scalar(den[:], ez[:], 1e37, 1.0, A.min, A.add)  # 1/p
                p_ = st("p")
                nc.vector.reciprocal(p_[:], den[:])
                om = st("om")
                nc.vector.tensor_scalar(om[:], p_[:], -1.0, 1.0, A.mult, A.add)  # 1-p
                om2 = st("om2")
                nc.vector.tensor_scalar(om2[:], om[:], 1e-38, None, A.max)
                lom = st("lom")
                nc.scalar.activation(lom[:], om2[:], AF.Ln)  # ln(1-p)
                q = st("q")
                nc.vector.tensor_scalar(q[:], den[:], THRESH, None, A.mult)
                # e2 = min(floor(q), 20) = sum_{n=1..20} [q >= n]
                acc = st("acc0")
                nc.vector.tensor_scalar(acc[:], q[:], 1.0, None, A.is_ge)
                for n in range(2, 21):
                    nxt = st(f"acc{n % 2}")
                    nc.vector.scalar_tensor_tensor(
                        nxt[:], q[:], float(n), acc[:], A.is_ge, A.add
                    )
                    acc = nxt
                e2 = acc
                h = st("h")
                nc.vector.tensor_scalar(h[:], e2[:], 19.5, None, A.is_lt)
                esp = st("esp")
                nc.vector.tensor_tensor(esp[:], e2[:], lom[:], A.mult)
                pw = st("pw")
                nc.scalar.activation(pw[:], esp[:], AF.Exp)  # (1-p)^e
                ep = st("ep")
                nc.vector.tensor_tensor(ep[:], e2[:], p_[:], A.mult)
                u = st("u")
                nc.vector.tensor_tensor(u[:], h[:], ep[:], A.mult)
                v = st("v")
                nc.vector.scalar_tensor_tensor(v[:], h[:], -1.0, u[:], A.mult, A.add)
                w2 = st("w2")
                nc.vector.tensor_tensor(w2[:], v[:], pw[:], A.mult)
                mm = st("mm")
                nc.vector.scalar_tensor_tensor(
                    mm[:], pw[:], -1.0, w2[:], A.mult, A.subtract
                )
                mb = st("mb")
                nc.vector.tensor_scalar(mb[:], mm[:], 1.0, None, A.add)

                a2 = st("a2")
                nc.vector.tensor_tensor(a2[:], e2[:], ep[:], A.subtract)
                b3 = st("b3")
                nc.vector.scalar_tensor_tensor(
                    b3[:], p_[:], 18.0, a2[:], A.add, A.subtract
                )
                c3 = st("c3")
                nc.vector.tensor_tensor(c3[:], h[:], b3[:], A.mult)
                pd = st("pd")
                nc.vector.tensor_scalar(pd[:], c3[:], -1.0, 20.0, A.mult, A.add)

                # --- scale x by m in place, store out ---
                for j in range(G):
                    t = g * G + j
                    nc.scalar.activation(
                        xts[j][:], xts[j][:], AF.Copy, scale=mb[:, j : j + 1]
                    )
                    nc.sync.dma_start(st_ext[bass.ts(t, P), :], xts[j][:])

                nc.sync.dma_start(pd_view[:, g * G : (g + 1) * G], pd[:])

    nc.compile()
    return nc


_NC_CACHE = []


def _get_nc():
    if not _NC_CACHE:
        _NC_CACHE.append(build_nc())
    return _NC_CACHE[0]


def make_in_maps(x, W, b):
    x = np.ascontiguousarray(x, dtype=np.float32)
    Wt = np.ascontiguousarray(
        np.broadcast_to(W.reshape(1, D), (P, D)), dtype=np.float32
    )
    nb = np.full((P, 1), -float(np.asarray(b).reshape(-1)[0]), dtype=np.float32)
    xs = x.reshape(N_CORES, ROWS, D)
    return [{"x": xs[c], "W": Wt, "nb": nb} for c in range(N_CORES)]


def kernel(x: np.ndarray, W: np.ndarray, b: np.ndarray):
    nc = _get_nc()
    in_maps = make_in_maps(x, W, b)
    res = run_bass_kernel_spmd(nc, in_maps, core_ids=list(range(N_CORES)))
    state = np.stack(
        [res.results[c]["out_state"] for c in range(N_CORES)], axis=0
    ).reshape(B, S, D)
    ponder = np.stack(
        [res.results[c]["out_ponder"] for c in range(N_CORES)], axis=0
    ).reshape(B, S, 1)
    return state, ponder
